# revision 18
# baseline (speedup 1.0000x reference)
"""3-layer GCN (message passing) on 8 Trainium2 NeuronCores.

Strategy
--------
Per GCN layer (using linearity: gcn(x) = (A_norm @ x) @ W + b):
  1. agg = A_norm @ h      -- sparse aggregate, done as per-dst-block PE matmuls
                              over dma_gather'ed source rows (fp16 table) with
                              host-built one-hot S matrices (fp8e4, segment sum).
  2. h' = ELU(agg @ W + b) -- dense GEMM per 128-node block + ELU epilogue.
Normalization dinv[src]*dinv[dst] is separable: the gather table holds
dinv-prescaled rows; dst-side dinv is applied as a per-partition ACT scale
when evicting the post-GEMM PSUM (h_ps rows are dst nodes).

Nodes are sharded contiguously across the 8 cores (6250 each); edges assigned
by destination core.  Between layers a G-chunk AllGather republishes the fp16
node-feature table; the table is laid out CHUNK-MAJOR (chunk g = contiguous
[8 cores x local rows r0_g:r1_g]) so each AG chunk's output is contiguous.
Each layer runs in G passes: pass g aggregates only edges whose source lies in
AG chunk g, so pass-g gathers depend only on AG chunk g -- the last AG chunk
is hidden behind pass 0..G-2 compute of the next layer.  Pass partials are
combined in SBUF (per-block fp32 staging).

Gather descriptor generation is the dominant cost, so gathers are spread
round-robin over 4 SWDGE queues (num_swdge_queues=4).

dma_gather indices are int16; each group's window (<= 8*3200 rows) is
addressed relative to the group's table base.
"""

import numpy as np

N = 50000
E = 800000
F = 128
H = 128
O = 64
NCORES = 8
NLOC = N // NCORES           # 6250
P = 128
NBLK = (NLOC + P - 1) // P   # 49, last block has 106 nodes
LAST_ROWS = NLOC - (NBLK - 1) * P   # 106

_CACHE = {}


def _ag_chunk_layout(ag_chunks):
    """Chunk-major table layout so each AllGather chunk's output is contiguous.

    pos(node i = c*NLOC + r) = obase[g] + c*size[g] + (r - r0[g]) where g is
    the AG chunk whose local row range [r0, r1) contains r.
    """
    bb = [round(g * NBLK / ag_chunks) for g in range(ag_chunks + 1)]
    row_rngs = [(bb[g] * P, min(bb[g + 1] * P, NLOC)) for g in range(ag_chunks)]
    sizes = [r1 - r0 for (r0, r1) in row_rngs]
    obase = np.concatenate([[0], np.cumsum(np.array(sizes) * NCORES)]).astype(np.int64)
    perm = np.empty(N, np.int64)
    r = np.arange(NLOC)
    for g, (r0, r1) in enumerate(row_rngs):
        for c in range(NCORES):
            perm[c * NLOC + r[r0:r1]] = obase[g] + c * sizes[g] + (r[r0:r1] - r0)
    return row_rngs, obase, perm


def _host_prep(x, edge_index):
    """Build per-core gather indices, S matrices, and scale vectors."""
    import os
    sfp8 = int(os.environ.get("GCN_SFP8", "1"))
    G = int(os.environ.get("GCN_AGCHUNKS", "2"))
    seqidx = int(os.environ.get("GCN_SEQIDX", "0"))
    row_rngs, obase, perm = _ag_chunk_layout(G)
    spans = [int(obase[g + 1] - obase[g]) for g in range(G)]
    assert max(spans) <= 32768

    src = np.ascontiguousarray(edge_index[0]).astype(np.int64)
    dst = np.ascontiguousarray(edge_index[1]).astype(np.int64)
    loops = np.arange(N, dtype=np.int64)
    src = np.concatenate([src, loops])
    dst = np.concatenate([dst, loops])

    deg = np.bincount(dst, minlength=N).astype(np.float64)  # includes self-loop
    dinv = (1.0 / np.sqrt(deg)).astype(np.float32)

    x_t = (dinv[:, None] * np.asarray(x, dtype=np.float32)).astype(np.float16)
    x_perm = np.empty_like(x_t)
    x_perm[perm] = x_t
    x_t = x_perm
    pos = perm[src]                     # gather addresses = table positions
    grp = np.searchsorted(obase[1:-1], pos, side="right")  # source AG chunk

    core = dst // NLOC
    ld = dst - core * NLOC
    blk = ld // P
    col = ld - blk * P

    key = (core * NBLK + blk) * G + grp
    order = np.argsort(key, kind="stable")
    pos_s = pos[order]
    key_s = key[order]
    col_s = col[order]
    grp_s = grp[order]

    cnt = np.bincount(key_s, minlength=NCORES * NBLK * G).reshape(NCORES, NBLK, G)
    # uniform (across cores) chunk counts per (block, group)
    Cg = np.max(np.ceil(cnt / P).astype(np.int64), axis=0)      # [NBLK, G]
    C = Cg.sum(axis=1)                                          # [NBLK]
    CT = int(C.sum())
    chunk_base = np.concatenate([[0], np.cumsum(C)]).astype(np.int64)
    # chunk offset of group g within block b
    gbase = np.zeros((NBLK, G), np.int64)
    for b in range(NBLK):
        gbase[b] = chunk_base[b] + np.concatenate([[0], np.cumsum(Cg[b])[:-1]])

    # per-edge slot assignment: rank within (core, blk, grp)
    kcnt = np.bincount(key_s, minlength=NCORES * NBLK * G)
    kstart = np.concatenate([[0], np.cumsum(kcnt)])
    rank = np.arange(pos_s.shape[0]) - kstart[key_s]
    blk_s = (key_s // G) % NBLK
    core_s = key_s // (G * NBLK)
    slot = gbase[blk_s, grp_s] * P + rank
    idx_val = pos_s - obase[grp_s]
    assert idx_val.min() >= 0 and idx_val.max() < 32768

    if sfp8:
        import ml_dtypes
        s_np_dtype = ml_dtypes.float8_e4m3
    else:
        s_np_dtype = np.float16

    idx_arrs = []
    S_arrs = []
    for c in range(NCORES):
        m = core_s == c
        sl = slot[m]
        ia = np.zeros(CT * P, np.int16)
        ia[sl] = idx_val[m].astype(np.int16)
        if seqidx:  # timing control: sequential addresses (wrong results)
            ia = (np.arange(CT * P) % 16384).astype(np.int16)
        cols = CT * P // 16
        w = np.zeros((cols, 16), np.int16)
        w.reshape(-1)[:] = ia
        idx_arrs.append(np.tile(w.T.copy(), (8, 1)))
        S = np.zeros((CT, P, P), s_np_dtype)
        S[sl // P, sl % P, col_s[m]] = 1.0
        S_arrs.append(np.ascontiguousarray(S.transpose(1, 0, 2)).reshape(P, CT * P))

    dinv_blk = []
    for c in range(NCORES):
        loc = np.zeros(NBLK * P, np.float32)
        loc[:NLOC] = dinv[c * NLOC:(c + 1) * NLOC]
        dinv_blk.append(np.ascontiguousarray(loc.reshape(NBLK, P).T))

    meta = dict(Cg=Cg.tolist(), C=C.tolist(), CT=CT, gbase=gbase.tolist(),
                chunk_base=chunk_base.tolist(), sfp8=sfp8, G=G,
                row_rngs=row_rngs, obase=obase.tolist(), spans=spans)
    return x_t, idx_arrs, S_arrs, dinv_blk, meta


def _build_program(meta, repeat=1):
    import os
    import concourse.mybir as mybir
    import concourse.tile as tile
    from concourse import bacc

    DBG_LAYERS = int(os.environ.get("GCN_LAYERS", "3"))
    DBG_AG = int(os.environ.get("GCN_AG", "1"))
    DBG_GATHERONLY = int(os.environ.get("GCN_GATHERONLY", "0"))
    NSWQ = int(os.environ.get("GCN_NSWQ", "4"))
    PASSES = int(os.environ.get("GCN_PASSES", "1"))

    Cg = meta["Cg"]
    C = meta["C"]
    CT = meta["CT"]
    chunk_base = meta["chunk_base"]
    gbase = meta["gbase"]
    sfp8 = meta["sfp8"]
    G = meta["G"]
    row_rngs = meta["row_rngs"]
    obase = meta["obase"]
    spans = meta["spans"]
    dt = mybir.dt
    s_dt = dt.float8e4 if sfp8 else dt.float16
    ALU = mybir.AluOpType
    ACTF = mybir.ActivationFunctionType

    nc = bacc.Bacc("TRN2", target_bir_lowering=False, num_devices=NCORES,
                   num_swdge_queues=NSWQ)

    t_xt = nc.dram_tensor("x_t", [N, F], dt.float16, kind="ExternalInput")
    t_idx = nc.dram_tensor("idx", [P, CT * 8], dt.int16, kind="ExternalInput")
    t_S = nc.dram_tensor("S", [P, CT * P], s_dt, kind="ExternalInput")
    t_dbk = nc.dram_tensor("dinv_blk", [P, NBLK], dt.float32, kind="ExternalInput")
    t_W = [nc.dram_tensor("W1", [F, H], dt.float16, kind="ExternalInput"),
           nc.dram_tensor("W2", [H, H], dt.float16, kind="ExternalInput"),
           nc.dram_tensor("W3", [H, O], dt.float16, kind="ExternalInput")]
    t_b = [nc.dram_tensor("b1_bc", [P, H], dt.float16, kind="ExternalInput"),
           nc.dram_tensor("b2_bc", [P, H], dt.float16, kind="ExternalInput"),
           nc.dram_tensor("b3_bc", [P, O], dt.float32, kind="ExternalInput")]
    t_out = nc.dram_tensor("out", [NLOC, O], dt.float32, kind="ExternalOutput")

    qctr = [0]

    def next_q():
        qctr[0] += 1
        return qctr[0] % NSWQ

    with tile.TileContext(nc) as tc:
        with (
            tc.tile_pool(name="const", bufs=1) as cpool,
            tc.tile_pool(name="gth", bufs=3) as gpool,
            tc.tile_pool(name="smat", bufs=3) as spool,
            tc.tile_pool(name="work", bufs=3) as wpool,
            tc.tile_pool(name="acc", bufs=1) as apool,
            tc.tile_pool(name="hout", bufs=3) as hpool,
            tc.tile_pool(name="psA", bufs=3, space="PSUM") as psA,
            tc.tile_pool(name="psH", bufs=2, space="PSUM") as psH,
            tc.tile_pool(name="dram", bufs=1, space="DRAM") as dpool,
        ):
            # constants
            idx_t = cpool.tile([P, CT * 8], dt.int16, tag="idx")
            nc.sync.dma_start(idx_t[:], t_idx[:, :])
            dbk_t = cpool.tile([P, NBLK], dt.float32, tag="dbk")
            nc.sync.dma_start(dbk_t[:], t_dbk[:, :])
            W_t = []
            b_t = []
            for l in range(3):
                wt = cpool.tile([128, t_W[l].shape[1]], dt.float16, tag=f"W{l}")
                nc.sync.dma_start(wt[:], t_W[l][:, :])
                W_t.append(wt)
                bt = cpool.tile([P, t_b[l].shape[1]],
                                dt.float16 if l < 2 else dt.float32, tag=f"b{l}")
                nc.sync.dma_start(bt[:], t_b[l][:, :])
                b_t.append(bt)

            # per-block partial aggregates (group partial staging)
            agg_sb = [apool.tile([P, P], dt.float32, tag=f"aggsb{b}",
                                 name=f"aggsb{b}")
                      for b in range(NBLK)]

            # inter-layer tables (internal DRAM)
            cc_in = [dpool.tile([NLOC, H], dt.float16, tag=f"ccin{l}", name=f"ccin{l}")
                     for l in range(2)]
            cc_out = [dpool.tile([N, H], dt.float16, tag=f"ccout{l}", name=f"ccout{l}")
                      for l in range(2)]

            def finish_block(l, b, agg, Hout):
                h_ps = psH.tile([P, Hout], dt.float32, tag="hps", name="h_ps")
                nc.tensor.matmul(out=h_ps[:, :], lhsT=agg[:], rhs=W_t[l][:, :],
                                 start=True, stop=True)
                rows = P if b < NBLK - 1 else LAST_ROWS
                if l < 2:
                    tmp = wpool.tile([P, Hout], dt.float16, tag="tmp", name="tmp")
                    nc.scalar.activation(out=tmp[:], in_=h_ps[:, :], func=ACTF.Copy,
                                         scale=dbk_t[:, b:b + 1])
                    t = wpool.tile([P, Hout], dt.float16, tag="t", name="t")
                    nc.vector.tensor_tensor(out=t[:], in0=tmp[:], in1=b_t[l][:, :],
                                            op=ALU.add)
                    # ELU(t) = max(exp(min(t,0)) - 1, t)
                    m = wpool.tile([P, Hout], dt.float16, tag="m", name="m")
                    nc.vector.tensor_scalar(out=m[:], in0=t[:], scalar1=0.0,
                                            scalar2=None, op0=ALU.min)
                    e = wpool.tile([P, Hout], dt.float16, tag="e", name="e")
                    nc.scalar.activation(out=e[:], in_=m[:], func=ACTF.Exp)
                    r = wpool.tile([P, Hout], dt.float16, tag="r", name="r")
                    nc.vector.tensor_scalar(out=r[:], in0=e[:], scalar1=-1.0,
                                            scalar2=None, op0=ALU.add)
                    s = wpool.tile([P, Hout], dt.float16, tag="s", name="s")
                    nc.vector.tensor_tensor(out=s[:], in0=r[:], in1=t[:], op=ALU.max)
                    # src-side dinv prescale for next layer's gather table
                    ht = hpool.tile([P, Hout], dt.float16, tag="ht", name="ht")
                    nc.scalar.activation(out=ht[:], in_=s[:], func=ACTF.Copy,
                                         scale=dbk_t[:, b:b + 1])
                    nc.sync.dma_start(cc_in[l][b * P:b * P + rows, :], ht[:rows, :])
                else:
                    tmp32 = wpool.tile([P, Hout], dt.float32, tag="tmp32", name="tmp32")
                    nc.scalar.activation(out=tmp32[:], in_=h_ps[:, :], func=ACTF.Copy,
                                         scale=dbk_t[:, b:b + 1])
                    t32 = wpool.tile([P, Hout], dt.float32, tag="t32", name="t32")
                    nc.vector.tensor_tensor(out=t32[:], in0=tmp32[:], in1=b_t[l][:, :],
                                            op=ALU.add)
                    nc.sync.dma_start(t_out[b * P:b * P + rows, :], t32[:rows, :])

            for rep in range(repeat):
              for l in range(DBG_LAYERS):
                tab = t_xt if l == 0 else cc_out[l - 1]
                Hout = H if l < 2 else O

                if PASSES == 1:
                    # fused: one pass over blocks; per block, one gather per
                    # group window, a single accumulation over all chunks.
                    for b in range(NBLK):
                        Cb = C[b]
                        cb0 = chunk_base[b]
                        g2 = gpool.tile([P, Cb * P], dt.float16, tag="g", name="g2")
                        g3 = g2[:, :].rearrange("p (c d) -> p c d", d=P)
                        for g in range(G):
                            Cbg = Cg[b][g]
                            if Cbg == 0:
                                continue
                            c0 = gbase[b][g] - cb0
                            nc.gpsimd.dma_gather(
                                out_ap=g3[:, c0:c0 + Cbg, :],
                                in_ap=tab[int(obase[g]):int(obase[g]) + spans[g], :],
                                idxs_ap=idx_t[:, (cb0 + c0) * 8:(cb0 + c0 + Cbg) * 8],
                                num_idxs=Cbg * P,
                                num_idxs_reg=Cbg * P,
                                elem_size=P,
                                single_packet=False,
                                queue_num=next_q(),
                            )
                        if DBG_GATHERONLY:
                            continue
                        S_t = spool.tile([P, Cb * P], s_dt, tag="S", name="S_t")
                        nc.sync.dma_start(S_t[:], t_S[:, cb0 * P:(cb0 + Cb) * P])
                        agg_ps = psA.tile([P, P], dt.float32, tag="aggps",
                                          name="agg_ps")
                        for c in range(Cb):
                            nc.tensor.matmul(
                                out=agg_ps[:, :],
                                lhsT=g2[:, c * P:(c + 1) * P],
                                rhs=S_t[:, c * P:(c + 1) * P],
                                start=(c == 0),
                                stop=(c == Cb - 1),
                            )
                        agg = wpool.tile([P, P], dt.float16, tag="agg", name="agg")
                        nc.scalar.activation(out=agg[:], in_=agg_ps[:, :],
                                             func=ACTF.Copy)
                        finish_block(l, b, agg, Hout)
                else:
                  for g in range(G):
                    win = tab[int(obase[g]):int(obase[g]) + spans[g], :]
                    for b in range(NBLK):
                        Cbg = Cg[b][g]
                        if Cbg == 0:
                            continue
                        cb0 = gbase[b][g]
                        g2 = gpool.tile([P, Cbg * P], dt.float16, tag=f"g{g}",
                                        name="g2")
                        g3 = g2[:, :].rearrange("p (c d) -> p c d", d=P)
                        nc.gpsimd.dma_gather(
                            out_ap=g3[:, 0:Cbg, :],
                            in_ap=win,
                            idxs_ap=idx_t[:, cb0 * 8:(cb0 + Cbg) * 8],
                            num_idxs=Cbg * P,
                            num_idxs_reg=Cbg * P,
                            elem_size=P,
                            single_packet=False,
                            queue_num=next_q(),
                        )
                        if DBG_GATHERONLY:
                            continue
                        S_t = spool.tile([P, Cbg * P], s_dt, tag=f"S{g}", name="S_t")
                        nc.sync.dma_start(S_t[:], t_S[:, cb0 * P:(cb0 + Cbg) * P])

                        agg_ps = psA.tile([P, P], dt.float32, tag="aggps",
                                          name="agg_ps")
                        for c in range(Cbg):
                            nc.tensor.matmul(
                                out=agg_ps[:, :],
                                lhsT=g2[:, c * P:(c + 1) * P],
                                rhs=S_t[:, c * P:(c + 1) * P],
                                start=(c == 0),
                                stop=(c == Cbg - 1),
                            )
                        if g == 0:
                            nc.scalar.activation(out=agg_sb[b][:], in_=agg_ps[:, :],
                                                 func=ACTF.Copy)
                            continue
                        elif g < G - 1:
                            nc.vector.tensor_tensor(out=agg_sb[b][:], in0=agg_ps[:, :],
                                                    in1=agg_sb[b][:], op=ALU.add)
                            continue
                        agg = wpool.tile([P, P], dt.float16, tag="agg", name="agg")
                        nc.vector.tensor_tensor(out=agg[:], in0=agg_ps[:, :],
                                                in1=agg_sb[b][:], op=ALU.add)
                        finish_block(l, b, agg, Hout)

                if l < 2 and l < DBG_LAYERS - 1 and DBG_AG and not DBG_GATHERONLY:
                    for g, (r0, r1) in enumerate(row_rngs):
                        o0, o1 = int(obase[g]), int(obase[g + 1])
                        nc.gpsimd.collective_compute(
                            "AllGather",
                            mybir.AluOpType.bypass,
                            replica_groups=[list(range(NCORES))],
                            ins=[cc_in[l][r0:r1, :].opt()],
                            outs=[cc_out[l][o0:o1, :].opt()],
                        )
    nc.compile()
    return nc


def _make_in_maps(np_inputs):
    x = np.asarray(np_inputs["x"])
    edge_index = np.asarray(np_inputs["edge_index"])
    x_t, idx_arrs, S_arrs, dinv_blk, meta = _host_prep(x, edge_index)

    b1_bc = np.ascontiguousarray(np.broadcast_to(
        np.asarray(np_inputs["b1"], np.float16)[None, :], (P, H)))
    b2_bc = np.ascontiguousarray(np.broadcast_to(
        np.asarray(np_inputs["b2"], np.float16)[None, :], (P, H)))
    b3_bc = np.ascontiguousarray(np.broadcast_to(
        np.asarray(np_inputs["b3"], np.float32)[None, :], (P, O)))
    W1 = np.ascontiguousarray(np.asarray(np_inputs["W1"], np.float32).astype(np.float16))
    W2 = np.ascontiguousarray(np.asarray(np_inputs["W2"], np.float32).astype(np.float16))
    W3 = np.ascontiguousarray(np.asarray(np_inputs["W3"], np.float32).astype(np.float16))

    in_maps = []
    for c in range(NCORES):
        in_maps.append({
            "x_t": x_t,
            "idx": idx_arrs[c],
            "S": S_arrs[c],
            "dinv_blk": dinv_blk[c],
            "W1": W1, "W2": W2, "W3": W3,
            "b1_bc": b1_bc, "b2_bc": b2_bc, "b3_bc": b3_bc,
        })
    return in_maps, meta


def kernel(x, edge_index, W1, b1, W2, b2, W3, b3):
    from concourse.bass_utils import run_bass_kernel_spmd

    in_maps, meta = _make_in_maps(dict(x=x, edge_index=edge_index, W1=W1, b1=b1,
                                       W2=W2, b2=b2, W3=W3, b3=b3))
    key = ("prog", meta["CT"], tuple(meta["C"]), meta["G"], 1)
    if key not in _CACHE:
        _CACHE[key] = _build_program(meta, repeat=1)
    nc = _CACHE[key]

    res = run_bass_kernel_spmd(nc, in_maps, core_ids=list(range(NCORES)))
    out = np.concatenate([res.results[c]["out"] for c in range(NCORES)], axis=0)
    return out.astype(np.float32)


# revision 24
# speedup vs baseline: 1.2601x; 1.2601x over previous
"""3-layer GCN (message passing) on 8 Trainium2 NeuronCores.

Strategy
--------
Per GCN layer (using linearity: gcn(x) = (A_norm @ x) @ W + b):
  1. agg = A_norm @ h      -- sparse aggregate, done as per-dst-block PE matmuls
                              over dma_gather'ed source rows (fp16 table) with
                              host-built one-hot S matrices (fp8e4, segment sum).
  2. h' = ELU(agg @ W + b) -- dense GEMM per 128-node block + ELU epilogue.
Normalization dinv[src]*dinv[dst] is separable: the gather table holds
dinv-prescaled rows; dst-side dinv is applied as a per-partition ACT scale
when evicting the post-GEMM PSUM (h_ps rows are dst nodes).

Nodes are sharded contiguously across the 8 cores (6250 each); edges assigned
by destination core.  Between layers a G-chunk AllGather republishes the fp16
node-feature table; the table is laid out CHUNK-MAJOR (chunk g = contiguous
[8 cores x local rows r0_g:r1_g]) so each AG chunk's output is contiguous.
Each layer runs in G passes: pass g aggregates only edges whose source lies in
AG chunk g, so pass-g gathers depend only on AG chunk g -- the last AG chunk
is hidden behind pass 0..G-2 compute of the next layer.  Pass partials are
combined in SBUF (per-block fp32 staging).

Gather descriptor generation is the dominant cost, so gathers are spread
round-robin over 4 SWDGE queues (num_swdge_queues=4).

dma_gather indices are int16; each group's window (<= 8*3200 rows) is
addressed relative to the group's table base.
"""

import numpy as np

N = 50000
E = 800000
F = 128
H = 128
O = 64
NCORES = 8
NLOC = N // NCORES           # 6250
P = 128
NBLK = (NLOC + P - 1) // P   # 49, last block has 106 nodes
LAST_ROWS = NLOC - (NBLK - 1) * P   # 106

_CACHE = {}


def _ag_chunk_layout(ag_chunks):
    """Chunk-major table layout so each AllGather chunk's output is contiguous.

    pos(node i = c*NLOC + r) = obase[g] + c*size[g] + (r - r0[g]) where g is
    the AG chunk whose local row range [r0, r1) contains r.
    """
    bb = [round(g * NBLK / ag_chunks) for g in range(ag_chunks + 1)]
    row_rngs = [(bb[g] * P, min(bb[g + 1] * P, NLOC)) for g in range(ag_chunks)]
    sizes = [r1 - r0 for (r0, r1) in row_rngs]
    obase = np.concatenate([[0], np.cumsum(np.array(sizes) * NCORES)]).astype(np.int64)
    perm = np.empty(N, np.int64)
    r = np.arange(NLOC)
    for g, (r0, r1) in enumerate(row_rngs):
        for c in range(NCORES):
            perm[c * NLOC + r[r0:r1]] = obase[g] + c * sizes[g] + (r[r0:r1] - r0)
    return row_rngs, obase, perm


def _host_prep(x, edge_index):
    """Build per-core gather indices, S matrices, and scale vectors."""
    import os
    sfp8 = int(os.environ.get("GCN_SFP8", "1"))
    G = int(os.environ.get("GCN_AGCHUNKS", "2"))
    seqidx = int(os.environ.get("GCN_SEQIDX", "0"))
    row_rngs, obase, perm = _ag_chunk_layout(G)
    spans = [int(obase[g + 1] - obase[g]) for g in range(G)]
    assert max(spans) <= 32768

    src = np.ascontiguousarray(edge_index[0]).astype(np.int64)
    dst = np.ascontiguousarray(edge_index[1]).astype(np.int64)
    loops = np.arange(N, dtype=np.int64)
    src = np.concatenate([src, loops])
    dst = np.concatenate([dst, loops])

    deg = np.bincount(dst, minlength=N).astype(np.float64)  # includes self-loop
    dinv = (1.0 / np.sqrt(deg)).astype(np.float32)

    x_t = (dinv[:, None] * np.asarray(x, dtype=np.float32)).astype(np.float16)
    x_perm = np.empty_like(x_t)
    x_perm[perm] = x_t
    x_t = x_perm
    pos = perm[src]                     # gather addresses = table positions

    core = dst // NLOC
    ld = dst - core * NLOC
    blk = ld // P
    col = ld - blk * P

    passes = int(os.environ.get("GCN_PASSES", "1"))
    if passes == 1:
        # Fused blocks wait for the whole table anyway, so gather windows need
        # not align with AG chunks.  Use two overlapping windows, lo = [0,
        # 32768) and hi = [N-32768, N), with flexible assignment of the
        # overlap so per-(core, block) chunk counts stay tight and uniform.
        HI_BASE = N - 32768
        cls = np.where(pos < HI_BASE, 0,
                       np.where(pos < 32768, 1, 2)).astype(np.int64)
        key = (core * NBLK + blk) * 4 + cls
        order = np.argsort(key, kind="stable")
        pos_s = pos[order]
        key_s = key[order]
        col_s = col[order]
        cnt = np.bincount(key_s, minlength=NCORES * NBLK * 4).reshape(
            NCORES, NBLK, 4)
        n_lo, n_fx, n_hi = cnt[:, :, 0], cnt[:, :, 1], cnt[:, :, 2]
        A = np.zeros(NBLK, np.int64)
        B = np.zeros(NBLK, np.int64)
        for b in range(NBLK):
            best = None
            a_min = int(np.max(np.ceil(n_lo[:, b] / P)))
            for a in range(a_min, a_min + 3):
                spill = np.maximum(0, n_fx[:, b] - (P * a - n_lo[:, b]))
                bb2 = int(np.max(np.ceil((n_hi[:, b] + spill) / P)))
                if best is None or a + bb2 < best[0] + best[1]:
                    best = (a, bb2)
            A[b], B[b] = best
        Cg = np.stack([A, B], axis=1)                           # [NBLK, 2]
        C = Cg.sum(axis=1)
        CT = int(C.sum())
        chunk_base = np.concatenate([[0], np.cumsum(C)]).astype(np.int64)
        gbase = np.stack([chunk_base[:-1], chunk_base[:-1] + A], axis=1)
        # per-edge slot assignment with flex spill, as in the original scheme
        grp2 = key_s >> 2
        grp_cnt = np.bincount(grp2, minlength=NCORES * NBLK)
        grp_start = np.concatenate([[0], np.cumsum(grp_cnt)])
        rank = np.arange(pos_s.shape[0]) - grp_start[grp2]
        core_s = grp2 // NBLK
        blk_s = grp2 % NBLK
        k_lo = np.minimum(n_lo + n_fx, P * A[None, :])
        k_lo_e = k_lo[core_s, blk_s]
        is_lo = rank < k_lo_e
        slot_in_blk = np.where(is_lo, rank, P * A[blk_s] + (rank - k_lo_e))
        slot = chunk_base[blk_s] * P + slot_in_blk
        idx_val = np.where(is_lo, pos_s, pos_s - HI_BASE)
        win_obase = [0, HI_BASE]
        win_spans = [32768, 32768]
    else:
        grp = np.searchsorted(obase[1:-1], pos, side="right")  # source AG chunk
        key = (core * NBLK + blk) * G + grp
        order = np.argsort(key, kind="stable")
        pos_s = pos[order]
        key_s = key[order]
        col_s = col[order]
        grp_s = grp[order]
        cnt = np.bincount(key_s, minlength=NCORES * NBLK * G).reshape(
            NCORES, NBLK, G)
        Cg = np.max(np.ceil(cnt / P).astype(np.int64), axis=0)      # [NBLK, G]
        C = Cg.sum(axis=1)
        CT = int(C.sum())
        chunk_base = np.concatenate([[0], np.cumsum(C)]).astype(np.int64)
        gbase = np.zeros((NBLK, G), np.int64)
        for b in range(NBLK):
            gbase[b] = chunk_base[b] + np.concatenate([[0], np.cumsum(Cg[b])[:-1]])
        kcnt = np.bincount(key_s, minlength=NCORES * NBLK * G)
        kstart = np.concatenate([[0], np.cumsum(kcnt)])
        rank = np.arange(pos_s.shape[0]) - kstart[key_s]
        blk_s = (key_s // G) % NBLK
        core_s = key_s // (G * NBLK)
        slot = gbase[blk_s, grp_s] * P + rank
        idx_val = pos_s - obase[grp_s]
        win_obase = [int(v) for v in obase[:-1]]
        win_spans = spans
    assert idx_val.min() >= 0 and idx_val.max() < 32768

    if sfp8:
        import ml_dtypes
        s_np_dtype = ml_dtypes.float8_e4m3
    else:
        s_np_dtype = np.float16

    idx_arrs = []
    S_arrs = []
    for c in range(NCORES):
        m = core_s == c
        sl = slot[m]
        ia = np.zeros(CT * P, np.int16)
        ia[sl] = idx_val[m].astype(np.int16)
        if seqidx:  # timing control: sequential addresses (wrong results)
            ia = (np.arange(CT * P) % 16384).astype(np.int16)
        cols = CT * P // 16
        w = np.zeros((cols, 16), np.int16)
        w.reshape(-1)[:] = ia
        idx_arrs.append(np.tile(w.T.copy(), (8, 1)))
        S = np.zeros((CT, P, P), s_np_dtype)
        S[sl // P, sl % P, col_s[m]] = 1.0
        S_arrs.append(np.ascontiguousarray(S.transpose(1, 0, 2)).reshape(P, CT * P))

    dinv_blk = []
    for c in range(NCORES):
        loc = np.zeros(NBLK * P, np.float32)
        loc[:NLOC] = dinv[c * NLOC:(c + 1) * NLOC]
        dinv_blk.append(np.ascontiguousarray(loc.reshape(NBLK, P).T))

    meta = dict(Cg=Cg.tolist(), C=C.tolist(), CT=CT, gbase=gbase.tolist(),
                chunk_base=chunk_base.tolist(), sfp8=sfp8, G=G,
                row_rngs=row_rngs, obase=obase.tolist(),
                win_obase=win_obase, win_spans=win_spans, passes=passes)
    return x_t, idx_arrs, S_arrs, dinv_blk, meta


def _build_program(meta, repeat=1):
    import os
    import concourse.mybir as mybir
    import concourse.tile as tile
    from concourse import bacc

    DBG_LAYERS = int(os.environ.get("GCN_LAYERS", "3"))
    DBG_AG = int(os.environ.get("GCN_AG", "1"))
    DBG_GATHERONLY = int(os.environ.get("GCN_GATHERONLY", "0"))
    NSWQ = int(os.environ.get("GCN_NSWQ", "4"))
    PASSES = meta["passes"]

    Cg = meta["Cg"]
    C = meta["C"]
    CT = meta["CT"]
    chunk_base = meta["chunk_base"]
    gbase = meta["gbase"]
    sfp8 = meta["sfp8"]
    row_rngs = meta["row_rngs"]
    obase = meta["obase"]
    win_obase = meta["win_obase"]
    win_spans = meta["win_spans"]
    NG = len(win_spans)
    dt = mybir.dt
    s_dt = dt.float8e4 if sfp8 else dt.float16
    ALU = mybir.AluOpType
    ACTF = mybir.ActivationFunctionType

    nc = bacc.Bacc("TRN2", target_bir_lowering=False, num_devices=NCORES,
                   num_swdge_queues=NSWQ)

    t_xt = nc.dram_tensor("x_t", [N, F], dt.float16, kind="ExternalInput")
    t_idx = nc.dram_tensor("idx", [P, CT * 8], dt.int16, kind="ExternalInput")
    t_S = nc.dram_tensor("S", [P, CT * P], s_dt, kind="ExternalInput")
    t_dbk = nc.dram_tensor("dinv_blk", [P, NBLK], dt.float32, kind="ExternalInput")
    t_W = [nc.dram_tensor("W1", [F, H], dt.float16, kind="ExternalInput"),
           nc.dram_tensor("W2", [H, H], dt.float16, kind="ExternalInput"),
           nc.dram_tensor("W3", [H, O], dt.float16, kind="ExternalInput")]
    t_b = [nc.dram_tensor("b1_bc", [P, H], dt.float16, kind="ExternalInput"),
           nc.dram_tensor("b2_bc", [P, H], dt.float16, kind="ExternalInput"),
           nc.dram_tensor("b3_bc", [P, O], dt.float32, kind="ExternalInput")]
    t_out = nc.dram_tensor("out", [NLOC, O], dt.float32, kind="ExternalOutput")

    qctr = [0]

    def next_q():
        qctr[0] += 1
        return qctr[0] % NSWQ

    with tile.TileContext(nc) as tc:
        with (
            tc.tile_pool(name="const", bufs=1) as cpool,
            tc.tile_pool(name="gth", bufs=3) as gpool,
            tc.tile_pool(name="smat", bufs=3) as spool,
            tc.tile_pool(name="work", bufs=3) as wpool,
            tc.tile_pool(name="acc", bufs=1) as apool,
            tc.tile_pool(name="hout", bufs=3) as hpool,
            tc.tile_pool(name="psA", bufs=3, space="PSUM") as psA,
            tc.tile_pool(name="psH", bufs=2, space="PSUM") as psH,
            tc.tile_pool(name="dram", bufs=1, space="DRAM") as dpool,
        ):
            # constants
            idx_t = cpool.tile([P, CT * 8], dt.int16, tag="idx")
            nc.sync.dma_start(idx_t[:], t_idx[:, :])
            dbk_t = cpool.tile([P, NBLK], dt.float32, tag="dbk")
            nc.sync.dma_start(dbk_t[:], t_dbk[:, :])
            W_t = []
            b_t = []
            for l in range(3):
                wt = cpool.tile([128, t_W[l].shape[1]], dt.float16, tag=f"W{l}")
                nc.sync.dma_start(wt[:], t_W[l][:, :])
                W_t.append(wt)
                bt = cpool.tile([P, t_b[l].shape[1]],
                                dt.float16 if l < 2 else dt.float32, tag=f"b{l}")
                nc.sync.dma_start(bt[:], t_b[l][:, :])
                b_t.append(bt)

            # per-block partial aggregates (group partial staging)
            agg_sb = [apool.tile([P, P], dt.float32, tag=f"aggsb{b}",
                                 name=f"aggsb{b}")
                      for b in range(NBLK)]

            # inter-layer tables (internal DRAM)
            cc_in = [dpool.tile([NLOC, H], dt.float16, tag=f"ccin{l}", name=f"ccin{l}")
                     for l in range(2)]
            cc_out = [dpool.tile([N, H], dt.float16, tag=f"ccout{l}", name=f"ccout{l}")
                      for l in range(2)]

            def finish_block(l, b, agg, Hout):
                h_ps = psH.tile([P, Hout], dt.float32, tag="hps", name="h_ps")
                nc.tensor.matmul(out=h_ps[:, :], lhsT=agg[:], rhs=W_t[l][:, :],
                                 start=True, stop=True)
                rows = P if b < NBLK - 1 else LAST_ROWS
                if l < 2:
                    tmp = wpool.tile([P, Hout], dt.float16, tag="tmp", name="tmp")
                    nc.scalar.activation(out=tmp[:], in_=h_ps[:, :], func=ACTF.Copy,
                                         scale=dbk_t[:, b:b + 1])
                    t = wpool.tile([P, Hout], dt.float16, tag="t", name="t")
                    nc.vector.tensor_tensor(out=t[:], in0=tmp[:], in1=b_t[l][:, :],
                                            op=ALU.add)
                    # ELU(t) = max(exp(min(t,0)) - 1, t)
                    m = wpool.tile([P, Hout], dt.float16, tag="m", name="m")
                    nc.vector.tensor_scalar(out=m[:], in0=t[:], scalar1=0.0,
                                            scalar2=None, op0=ALU.min)
                    e = wpool.tile([P, Hout], dt.float16, tag="e", name="e")
                    nc.scalar.activation(out=e[:], in_=m[:], func=ACTF.Exp)
                    r = wpool.tile([P, Hout], dt.float16, tag="r", name="r")
                    nc.vector.tensor_scalar(out=r[:], in0=e[:], scalar1=-1.0,
                                            scalar2=None, op0=ALU.add)
                    s = wpool.tile([P, Hout], dt.float16, tag="s", name="s")
                    nc.vector.tensor_tensor(out=s[:], in0=r[:], in1=t[:], op=ALU.max)
                    # src-side dinv prescale for next layer's gather table
                    ht = hpool.tile([P, Hout], dt.float16, tag="ht", name="ht")
                    nc.scalar.activation(out=ht[:], in_=s[:], func=ACTF.Copy,
                                         scale=dbk_t[:, b:b + 1])
                    nc.sync.dma_start(cc_in[l][b * P:b * P + rows, :], ht[:rows, :])
                else:
                    tmp32 = wpool.tile([P, Hout], dt.float32, tag="tmp32", name="tmp32")
                    nc.scalar.activation(out=tmp32[:], in_=h_ps[:, :], func=ACTF.Copy,
                                         scale=dbk_t[:, b:b + 1])
                    t32 = wpool.tile([P, Hout], dt.float32, tag="t32", name="t32")
                    nc.vector.tensor_tensor(out=t32[:], in0=tmp32[:], in1=b_t[l][:, :],
                                            op=ALU.add)
                    nc.sync.dma_start(t_out[b * P:b * P + rows, :], t32[:rows, :])

            for rep in range(repeat):
              for l in range(DBG_LAYERS):
                tab = t_xt if l == 0 else cc_out[l - 1]
                Hout = H if l < 2 else O

                if PASSES == 1:
                    # fused: one pass over blocks; per block, one gather per
                    # group window, a single accumulation over all chunks.
                    for b in range(NBLK):
                        Cb = C[b]
                        cb0 = chunk_base[b]
                        g2 = gpool.tile([P, Cb * P], dt.float16, tag="g", name="g2")
                        g3 = g2[:, :].rearrange("p (c d) -> p c d", d=P)
                        for g in range(NG):
                            Cbg = Cg[b][g]
                            if Cbg == 0:
                                continue
                            c0 = gbase[b][g] - cb0
                            nc.gpsimd.dma_gather(
                                out_ap=g3[:, c0:c0 + Cbg, :],
                                in_ap=tab[win_obase[g]:win_obase[g] + win_spans[g], :],
                                idxs_ap=idx_t[:, (cb0 + c0) * 8:(cb0 + c0 + Cbg) * 8],
                                num_idxs=Cbg * P,
                                num_idxs_reg=Cbg * P,
                                elem_size=P,
                                single_packet=False,
                                queue_num=next_q(),
                            )
                        if DBG_GATHERONLY:
                            continue
                        S_t = spool.tile([P, Cb * P], s_dt, tag="S", name="S_t")
                        nc.sync.dma_start(S_t[:], t_S[:, cb0 * P:(cb0 + Cb) * P])
                        agg_ps = psA.tile([P, P], dt.float32, tag="aggps",
                                          name="agg_ps")
                        for c in range(Cb):
                            nc.tensor.matmul(
                                out=agg_ps[:, :],
                                lhsT=g2[:, c * P:(c + 1) * P],
                                rhs=S_t[:, c * P:(c + 1) * P],
                                start=(c == 0),
                                stop=(c == Cb - 1),
                            )
                        agg = wpool.tile([P, P], dt.float16, tag="agg", name="agg")
                        nc.scalar.activation(out=agg[:], in_=agg_ps[:, :],
                                             func=ACTF.Copy)
                        finish_block(l, b, agg, Hout)
                else:
                  for g in range(NG):
                    win = tab[win_obase[g]:win_obase[g] + win_spans[g], :]
                    for b in range(NBLK):
                        Cbg = Cg[b][g]
                        if Cbg == 0:
                            continue
                        cb0 = gbase[b][g]
                        g2 = gpool.tile([P, Cbg * P], dt.float16, tag=f"g{g}",
                                        name="g2")
                        g3 = g2[:, :].rearrange("p (c d) -> p c d", d=P)
                        nc.gpsimd.dma_gather(
                            out_ap=g3[:, 0:Cbg, :],
                            in_ap=win,
                            idxs_ap=idx_t[:, cb0 * 8:(cb0 + Cbg) * 8],
                            num_idxs=Cbg * P,
                            num_idxs_reg=Cbg * P,
                            elem_size=P,
                            single_packet=False,
                            queue_num=next_q(),
                        )
                        if DBG_GATHERONLY:
                            continue
                        S_t = spool.tile([P, Cbg * P], s_dt, tag=f"S{g}", name="S_t")
                        nc.sync.dma_start(S_t[:], t_S[:, cb0 * P:(cb0 + Cbg) * P])

                        agg_ps = psA.tile([P, P], dt.float32, tag="aggps",
                                          name="agg_ps")
                        for c in range(Cbg):
                            nc.tensor.matmul(
                                out=agg_ps[:, :],
                                lhsT=g2[:, c * P:(c + 1) * P],
                                rhs=S_t[:, c * P:(c + 1) * P],
                                start=(c == 0),
                                stop=(c == Cbg - 1),
                            )
                        if g == 0:
                            nc.scalar.activation(out=agg_sb[b][:], in_=agg_ps[:, :],
                                                 func=ACTF.Copy)
                            continue
                        elif g < NG - 1:
                            nc.vector.tensor_tensor(out=agg_sb[b][:], in0=agg_ps[:, :],
                                                    in1=agg_sb[b][:], op=ALU.add)
                            continue
                        agg = wpool.tile([P, P], dt.float16, tag="agg", name="agg")
                        nc.vector.tensor_tensor(out=agg[:], in0=agg_ps[:, :],
                                                in1=agg_sb[b][:], op=ALU.add)
                        finish_block(l, b, agg, Hout)

                if l < 2 and l < DBG_LAYERS - 1 and DBG_AG and not DBG_GATHERONLY:
                    for g, (r0, r1) in enumerate(row_rngs):
                        o0, o1 = int(obase[g]), int(obase[g + 1])
                        nc.gpsimd.collective_compute(
                            "AllGather",
                            mybir.AluOpType.bypass,
                            replica_groups=[list(range(NCORES))],
                            ins=[cc_in[l][r0:r1, :].opt()],
                            outs=[cc_out[l][o0:o1, :].opt()],
                        )
    nc.compile()
    return nc


def _make_in_maps(np_inputs):
    x = np.asarray(np_inputs["x"])
    edge_index = np.asarray(np_inputs["edge_index"])
    x_t, idx_arrs, S_arrs, dinv_blk, meta = _host_prep(x, edge_index)

    b1_bc = np.ascontiguousarray(np.broadcast_to(
        np.asarray(np_inputs["b1"], np.float16)[None, :], (P, H)))
    b2_bc = np.ascontiguousarray(np.broadcast_to(
        np.asarray(np_inputs["b2"], np.float16)[None, :], (P, H)))
    b3_bc = np.ascontiguousarray(np.broadcast_to(
        np.asarray(np_inputs["b3"], np.float32)[None, :], (P, O)))
    W1 = np.ascontiguousarray(np.asarray(np_inputs["W1"], np.float32).astype(np.float16))
    W2 = np.ascontiguousarray(np.asarray(np_inputs["W2"], np.float32).astype(np.float16))
    W3 = np.ascontiguousarray(np.asarray(np_inputs["W3"], np.float32).astype(np.float16))

    in_maps = []
    for c in range(NCORES):
        in_maps.append({
            "x_t": x_t,
            "idx": idx_arrs[c],
            "S": S_arrs[c],
            "dinv_blk": dinv_blk[c],
            "W1": W1, "W2": W2, "W3": W3,
            "b1_bc": b1_bc, "b2_bc": b2_bc, "b3_bc": b3_bc,
        })
    return in_maps, meta


def kernel(x, edge_index, W1, b1, W2, b2, W3, b3):
    from concourse.bass_utils import run_bass_kernel_spmd

    in_maps, meta = _make_in_maps(dict(x=x, edge_index=edge_index, W1=W1, b1=b1,
                                       W2=W2, b2=b2, W3=W3, b3=b3))
    key = ("prog", meta["CT"], tuple(meta["C"]), meta["G"], 1)
    if key not in _CACHE:
        _CACHE[key] = _build_program(meta, repeat=1)
    nc = _CACHE[key]

    res = run_bass_kernel_spmd(nc, in_maps, core_ids=list(range(NCORES)))
    out = np.concatenate([res.results[c]["out"] for c in range(NCORES)], axis=0)
    return out.astype(np.float32)


# revision 27
# speedup vs baseline: 1.3261x; 1.0524x over previous
"""3-layer GCN (message passing) on 8 Trainium2 NeuronCores.

Strategy
--------
Per GCN layer (using linearity: gcn(x) = (A_norm @ x) @ W + b):
  1. agg = A_norm @ h      -- sparse aggregate, done as per-dst-block PE matmuls
                              over dma_gather'ed source rows (fp16 table) with
                              host-built one-hot S matrices (fp8e4, segment sum).
  2. h' = ELU(agg @ W + b) -- dense GEMM per 128-node block + ELU epilogue.
Normalization dinv[src]*dinv[dst] is separable: the gather table holds
dinv-prescaled rows; dst-side dinv is applied as a per-partition ACT scale
when evicting the post-GEMM PSUM (h_ps rows are dst nodes).

Nodes are sharded contiguously across the 8 cores (6250 each); edges assigned
by destination core.  Between layers a 2-chunk AllGather republishes the fp16
node-feature table; the table is laid out CHUNK-MAJOR (chunk g = contiguous
[8 cores x local rows r0_g:r1_g]) so each AG chunk's output is contiguous and
overlaps trailing-block compute.

Gather descriptor generation is the dominant cost, so gathers are spread
round-robin over 4 SWDGE queues (num_swdge_queues=4) -- ~10x over one queue.

dma_gather indices are int16; each AG chunk's table range (<= 25600 rows) is
a gather window addressed relative to the chunk base, so per (dst block,
window) slot runs pad to 128-slot chunks uniformly across cores (single SPMD
program).
"""

import numpy as np

N = 50000
E = 800000
F = 128
H = 128
O = 64
NCORES = 8
NLOC = N // NCORES           # 6250
P = 128
NBLK = (NLOC + P - 1) // P   # 49, last block has 106 nodes
LAST_ROWS = NLOC - (NBLK - 1) * P   # 106

_CACHE = {}


def _ag_chunk_layout(ag_chunks):
    """Chunk-major table layout so each AllGather chunk's output is contiguous.

    pos(node i = c*NLOC + r) = obase[g] + c*size[g] + (r - r0[g]) where g is
    the AG chunk whose local row range [r0, r1) contains r.
    """
    bb = [round(g * NBLK / ag_chunks) for g in range(ag_chunks + 1)]
    row_rngs = [(bb[g] * P, min(bb[g + 1] * P, NLOC)) for g in range(ag_chunks)]
    sizes = [r1 - r0 for (r0, r1) in row_rngs]
    obase = np.concatenate([[0], np.cumsum(np.array(sizes) * NCORES)]).astype(np.int64)
    perm = np.empty(N, np.int64)
    r = np.arange(NLOC)
    for g, (r0, r1) in enumerate(row_rngs):
        for c in range(NCORES):
            perm[c * NLOC + r[r0:r1]] = obase[g] + c * sizes[g] + (r[r0:r1] - r0)
    return row_rngs, obase, perm


def _host_prep(x, edge_index):
    """Build per-core gather indices, S matrices, and scale vectors."""
    import os
    sfp8 = int(os.environ.get("GCN_SFP8", "1"))
    G = int(os.environ.get("GCN_AGCHUNKS", "2"))
    seqidx = int(os.environ.get("GCN_SEQIDX", "0"))
    row_rngs, obase, perm = _ag_chunk_layout(G)
    spans = [int(obase[g + 1] - obase[g]) for g in range(G)]
    assert max(spans) <= 32768

    src = np.ascontiguousarray(edge_index[0]).astype(np.int64)
    dst = np.ascontiguousarray(edge_index[1]).astype(np.int64)
    loops = np.arange(N, dtype=np.int64)
    src = np.concatenate([src, loops])
    dst = np.concatenate([dst, loops])

    deg = np.bincount(dst, minlength=N).astype(np.float64)  # includes self-loop
    dinv = (1.0 / np.sqrt(deg)).astype(np.float32)

    x_t = (dinv[:, None] * np.asarray(x, dtype=np.float32)).astype(np.float16)
    x_perm = np.empty_like(x_t)
    x_perm[perm] = x_t
    x_t = x_perm
    pos = perm[src]                     # gather addresses = table positions

    core = dst // NLOC
    ld = dst - core * NLOC
    blk = ld // P
    col = ld - blk * P

    passes = int(os.environ.get("GCN_PASSES", "1"))
    flexwin = int(os.environ.get("GCN_FLEXWIN", "0"))
    if passes == 1 and flexwin:
        # Fused blocks wait for the whole table anyway, so gather windows need
        # not align with AG chunks.  Use two overlapping windows, lo = [0,
        # 32768) and hi = [N-32768, N), with flexible assignment of the
        # overlap so per-(core, block) chunk counts stay tight and uniform.
        HI_BASE = N - 32768
        cls = np.where(pos < HI_BASE, 0,
                       np.where(pos < 32768, 1, 2)).astype(np.int64)
        key = (core * NBLK + blk) * 4 + cls
        order = np.argsort(key, kind="stable")
        pos_s = pos[order]
        key_s = key[order]
        col_s = col[order]
        cnt = np.bincount(key_s, minlength=NCORES * NBLK * 4).reshape(
            NCORES, NBLK, 4)
        n_lo, n_fx, n_hi = cnt[:, :, 0], cnt[:, :, 1], cnt[:, :, 2]
        A = np.zeros(NBLK, np.int64)
        B = np.zeros(NBLK, np.int64)
        for b in range(NBLK):
            best = None
            a_min = int(np.max(np.ceil(n_lo[:, b] / P)))
            for a in range(a_min, a_min + 3):
                spill = np.maximum(0, n_fx[:, b] - (P * a - n_lo[:, b]))
                bb2 = int(np.max(np.ceil((n_hi[:, b] + spill) / P)))
                if best is None or a + bb2 < best[0] + best[1]:
                    best = (a, bb2)
            A[b], B[b] = best
        Cg = np.stack([A, B], axis=1)                           # [NBLK, 2]
        C = Cg.sum(axis=1)
        CT = int(C.sum())
        chunk_base = np.concatenate([[0], np.cumsum(C)]).astype(np.int64)
        gbase = np.stack([chunk_base[:-1], chunk_base[:-1] + A], axis=1)
        # per-edge slot assignment with flex spill, as in the original scheme
        grp2 = key_s >> 2
        grp_cnt = np.bincount(grp2, minlength=NCORES * NBLK)
        grp_start = np.concatenate([[0], np.cumsum(grp_cnt)])
        rank = np.arange(pos_s.shape[0]) - grp_start[grp2]
        core_s = grp2 // NBLK
        blk_s = grp2 % NBLK
        k_lo = np.minimum(n_lo + n_fx, P * A[None, :])
        k_lo_e = k_lo[core_s, blk_s]
        is_lo = rank < k_lo_e
        slot_in_blk = np.where(is_lo, rank, P * A[blk_s] + (rank - k_lo_e))
        slot = chunk_base[blk_s] * P + slot_in_blk
        idx_val = np.where(is_lo, pos_s, pos_s - HI_BASE)
        win_obase = [0, HI_BASE]
        win_spans = [32768, 32768]
    else:
        grp = np.searchsorted(obase[1:-1], pos, side="right")  # source AG chunk
        key = (core * NBLK + blk) * G + grp
        order = np.argsort(key, kind="stable")
        pos_s = pos[order]
        key_s = key[order]
        col_s = col[order]
        grp_s = grp[order]
        cnt = np.bincount(key_s, minlength=NCORES * NBLK * G).reshape(
            NCORES, NBLK, G)
        Cg = np.max(np.ceil(cnt / P).astype(np.int64), axis=0)      # [NBLK, G]
        C = Cg.sum(axis=1)
        CT = int(C.sum())
        chunk_base = np.concatenate([[0], np.cumsum(C)]).astype(np.int64)
        gbase = np.zeros((NBLK, G), np.int64)
        for b in range(NBLK):
            gbase[b] = chunk_base[b] + np.concatenate([[0], np.cumsum(Cg[b])[:-1]])
        kcnt = np.bincount(key_s, minlength=NCORES * NBLK * G)
        kstart = np.concatenate([[0], np.cumsum(kcnt)])
        rank = np.arange(pos_s.shape[0]) - kstart[key_s]
        blk_s = (key_s // G) % NBLK
        core_s = key_s // (G * NBLK)
        slot = gbase[blk_s, grp_s] * P + rank
        idx_val = pos_s - obase[grp_s]
        win_obase = [int(v) for v in obase[:-1]]
        win_spans = spans
    assert idx_val.min() >= 0 and idx_val.max() < 32768

    if sfp8:
        import ml_dtypes
        s_np_dtype = ml_dtypes.float8_e4m3
    else:
        s_np_dtype = np.float16

    idx_arrs = []
    S_arrs = []
    for c in range(NCORES):
        m = core_s == c
        sl = slot[m]
        ia = np.zeros(CT * P, np.int16)
        ia[sl] = idx_val[m].astype(np.int16)
        if seqidx:  # timing control: sequential addresses (wrong results)
            ia = (np.arange(CT * P) % 16384).astype(np.int16)
        cols = CT * P // 16
        w = np.zeros((cols, 16), np.int16)
        w.reshape(-1)[:] = ia
        idx_arrs.append(np.tile(w.T.copy(), (8, 1)))
        S = np.zeros((CT, P, P), s_np_dtype)
        S[sl // P, sl % P, col_s[m]] = 1.0
        S_arrs.append(np.ascontiguousarray(S.transpose(1, 0, 2)).reshape(P, CT * P))

    dinv_blk = []
    for c in range(NCORES):
        loc = np.zeros(NBLK * P, np.float32)
        loc[:NLOC] = dinv[c * NLOC:(c + 1) * NLOC]
        dinv_blk.append(np.ascontiguousarray(loc.reshape(NBLK, P).T))

    meta = dict(Cg=Cg.tolist(), C=C.tolist(), CT=CT, gbase=gbase.tolist(),
                chunk_base=chunk_base.tolist(), sfp8=sfp8, G=G,
                row_rngs=row_rngs, obase=obase.tolist(),
                win_obase=win_obase, win_spans=win_spans, passes=passes)
    return x_t, idx_arrs, S_arrs, dinv_blk, meta


def _build_program(meta, repeat=1):
    import os
    import concourse.mybir as mybir
    import concourse.tile as tile
    from concourse import bacc

    DBG_LAYERS = int(os.environ.get("GCN_LAYERS", "3"))
    DBG_AG = int(os.environ.get("GCN_AG", "1"))
    DBG_GATHERONLY = int(os.environ.get("GCN_GATHERONLY", "0"))
    NSWQ = int(os.environ.get("GCN_NSWQ", "4"))
    PASSES = meta["passes"]

    Cg = meta["Cg"]
    C = meta["C"]
    CT = meta["CT"]
    chunk_base = meta["chunk_base"]
    gbase = meta["gbase"]
    sfp8 = meta["sfp8"]
    row_rngs = meta["row_rngs"]
    obase = meta["obase"]
    win_obase = meta["win_obase"]
    win_spans = meta["win_spans"]
    NG = len(win_spans)
    dt = mybir.dt
    s_dt = dt.float8e4 if sfp8 else dt.float16
    ALU = mybir.AluOpType
    ACTF = mybir.ActivationFunctionType

    nc = bacc.Bacc("TRN2", target_bir_lowering=False, num_devices=NCORES,
                   num_swdge_queues=NSWQ)

    t_xt = nc.dram_tensor("x_t", [N, F], dt.float16, kind="ExternalInput")
    t_idx = nc.dram_tensor("idx", [P, CT * 8], dt.int16, kind="ExternalInput")
    t_S = nc.dram_tensor("S", [P, CT * P], s_dt, kind="ExternalInput")
    t_dbk = nc.dram_tensor("dinv_blk", [P, NBLK], dt.float32, kind="ExternalInput")
    t_W = [nc.dram_tensor("W1", [F, H], dt.float16, kind="ExternalInput"),
           nc.dram_tensor("W2", [H, H], dt.float16, kind="ExternalInput"),
           nc.dram_tensor("W3", [H, O], dt.float16, kind="ExternalInput")]
    t_b = [nc.dram_tensor("b1_bc", [P, H], dt.float16, kind="ExternalInput"),
           nc.dram_tensor("b2_bc", [P, H], dt.float16, kind="ExternalInput"),
           nc.dram_tensor("b3_bc", [P, O], dt.float32, kind="ExternalInput")]
    t_out = nc.dram_tensor("out", [NLOC, O], dt.float32, kind="ExternalOutput")

    qctr = [0]

    def next_q():
        qctr[0] += 1
        return qctr[0] % NSWQ

    with tile.TileContext(nc) as tc:
        with (
            tc.tile_pool(name="const", bufs=1) as cpool,
            tc.tile_pool(name="gth", bufs=3) as gpool,
            tc.tile_pool(name="smat", bufs=3) as spool,
            tc.tile_pool(name="work", bufs=3) as wpool,
            tc.tile_pool(name="acc", bufs=1) as apool,
            tc.tile_pool(name="hout", bufs=3) as hpool,
            tc.tile_pool(name="psA", bufs=3, space="PSUM") as psA,
            tc.tile_pool(name="psH", bufs=2, space="PSUM") as psH,
            tc.tile_pool(name="dram", bufs=1, space="DRAM") as dpool,
        ):
            # constants
            idx_t = cpool.tile([P, CT * 8], dt.int16, tag="idx")
            nc.sync.dma_start(idx_t[:], t_idx[:, :])
            dbk_t = cpool.tile([P, NBLK], dt.float32, tag="dbk")
            nc.sync.dma_start(dbk_t[:], t_dbk[:, :])
            W_t = []
            b_t = []
            for l in range(3):
                wt = cpool.tile([128, t_W[l].shape[1]], dt.float16, tag=f"W{l}")
                nc.sync.dma_start(wt[:], t_W[l][:, :])
                W_t.append(wt)
                bt = cpool.tile([P, t_b[l].shape[1]],
                                dt.float16 if l < 2 else dt.float32, tag=f"b{l}")
                nc.sync.dma_start(bt[:], t_b[l][:, :])
                b_t.append(bt)

            # per-block partial aggregates (group partial staging)
            agg_sb = [apool.tile([P, P], dt.float32, tag=f"aggsb{b}",
                                 name=f"aggsb{b}")
                      for b in range(NBLK)]

            # inter-layer tables (internal DRAM)
            cc_in = [dpool.tile([NLOC, H], dt.float16, tag=f"ccin{l}", name=f"ccin{l}")
                     for l in range(2)]
            cc_out = [dpool.tile([N, H], dt.float16, tag=f"ccout{l}", name=f"ccout{l}")
                      for l in range(2)]

            def finish_block(l, b, agg, Hout):
                h_ps = psH.tile([P, Hout], dt.float32, tag="hps", name="h_ps")
                nc.tensor.matmul(out=h_ps[:, :], lhsT=agg[:], rhs=W_t[l][:, :],
                                 start=True, stop=True)
                rows = P if b < NBLK - 1 else LAST_ROWS
                if l < 2:
                    tmp = wpool.tile([P, Hout], dt.float16, tag="tmp", name="tmp")
                    nc.scalar.activation(out=tmp[:], in_=h_ps[:, :], func=ACTF.Copy,
                                         scale=dbk_t[:, b:b + 1])
                    t = wpool.tile([P, Hout], dt.float16, tag="t", name="t")
                    nc.vector.tensor_tensor(out=t[:], in0=tmp[:], in1=b_t[l][:, :],
                                            op=ALU.add)
                    # ELU(t) = max(exp(min(t,0)) - 1, t)
                    m = wpool.tile([P, Hout], dt.float16, tag="m", name="m")
                    nc.vector.tensor_scalar(out=m[:], in0=t[:], scalar1=0.0,
                                            scalar2=None, op0=ALU.min)
                    e = wpool.tile([P, Hout], dt.float16, tag="e", name="e")
                    nc.scalar.activation(out=e[:], in_=m[:], func=ACTF.Exp)
                    r = wpool.tile([P, Hout], dt.float16, tag="r", name="r")
                    nc.vector.tensor_scalar(out=r[:], in0=e[:], scalar1=-1.0,
                                            scalar2=None, op0=ALU.add)
                    s = wpool.tile([P, Hout], dt.float16, tag="s", name="s")
                    nc.vector.tensor_tensor(out=s[:], in0=r[:], in1=t[:], op=ALU.max)
                    # src-side dinv prescale for next layer's gather table
                    ht = hpool.tile([P, Hout], dt.float16, tag="ht", name="ht")
                    nc.scalar.activation(out=ht[:], in_=s[:], func=ACTF.Copy,
                                         scale=dbk_t[:, b:b + 1])
                    nc.sync.dma_start(cc_in[l][b * P:b * P + rows, :], ht[:rows, :])
                else:
                    tmp32 = wpool.tile([P, Hout], dt.float32, tag="tmp32", name="tmp32")
                    nc.scalar.activation(out=tmp32[:], in_=h_ps[:, :], func=ACTF.Copy,
                                         scale=dbk_t[:, b:b + 1])
                    t32 = wpool.tile([P, Hout], dt.float32, tag="t32", name="t32")
                    nc.vector.tensor_tensor(out=t32[:], in0=tmp32[:], in1=b_t[l][:, :],
                                            op=ALU.add)
                    nc.sync.dma_start(t_out[b * P:b * P + rows, :], t32[:rows, :])

            for rep in range(repeat):
              for l in range(DBG_LAYERS):
                tab = t_xt if l == 0 else cc_out[l - 1]
                Hout = H if l < 2 else O

                if PASSES == 1:
                    # fused: one pass over blocks; per block, one gather per
                    # group window, a single accumulation over all chunks.
                    for b in range(NBLK):
                        Cb = C[b]
                        cb0 = chunk_base[b]
                        g2 = gpool.tile([P, Cb * P], dt.float16, tag="g", name="g2")
                        g3 = g2[:, :].rearrange("p (c d) -> p c d", d=P)
                        for g in range(NG):
                            Cbg = Cg[b][g]
                            if Cbg == 0:
                                continue
                            c0 = gbase[b][g] - cb0
                            nc.gpsimd.dma_gather(
                                out_ap=g3[:, c0:c0 + Cbg, :],
                                in_ap=tab[win_obase[g]:win_obase[g] + win_spans[g], :],
                                idxs_ap=idx_t[:, (cb0 + c0) * 8:(cb0 + c0 + Cbg) * 8],
                                num_idxs=Cbg * P,
                                num_idxs_reg=Cbg * P,
                                elem_size=P,
                                single_packet=False,
                                queue_num=next_q(),
                            )
                        if DBG_GATHERONLY:
                            continue
                        S_t = spool.tile([P, Cb * P], s_dt, tag="S", name="S_t")
                        nc.sync.dma_start(S_t[:], t_S[:, cb0 * P:(cb0 + Cb) * P])
                        agg_ps = psA.tile([P, P], dt.float32, tag="aggps",
                                          name="agg_ps")
                        for c in range(Cb):
                            nc.tensor.matmul(
                                out=agg_ps[:, :],
                                lhsT=g2[:, c * P:(c + 1) * P],
                                rhs=S_t[:, c * P:(c + 1) * P],
                                start=(c == 0),
                                stop=(c == Cb - 1),
                            )
                        agg = wpool.tile([P, P], dt.float16, tag="agg", name="agg")
                        nc.scalar.activation(out=agg[:], in_=agg_ps[:, :],
                                             func=ACTF.Copy)
                        finish_block(l, b, agg, Hout)
                else:
                  for g in range(NG):
                    win = tab[win_obase[g]:win_obase[g] + win_spans[g], :]
                    for b in range(NBLK):
                        Cbg = Cg[b][g]
                        if Cbg == 0:
                            continue
                        cb0 = gbase[b][g]
                        g2 = gpool.tile([P, Cbg * P], dt.float16, tag=f"g{g}",
                                        name="g2")
                        g3 = g2[:, :].rearrange("p (c d) -> p c d", d=P)
                        nc.gpsimd.dma_gather(
                            out_ap=g3[:, 0:Cbg, :],
                            in_ap=win,
                            idxs_ap=idx_t[:, cb0 * 8:(cb0 + Cbg) * 8],
                            num_idxs=Cbg * P,
                            num_idxs_reg=Cbg * P,
                            elem_size=P,
                            single_packet=False,
                            queue_num=next_q(),
                        )
                        if DBG_GATHERONLY:
                            continue
                        S_t = spool.tile([P, Cbg * P], s_dt, tag=f"S{g}", name="S_t")
                        nc.sync.dma_start(S_t[:], t_S[:, cb0 * P:(cb0 + Cbg) * P])

                        agg_ps = psA.tile([P, P], dt.float32, tag="aggps",
                                          name="agg_ps")
                        for c in range(Cbg):
                            nc.tensor.matmul(
                                out=agg_ps[:, :],
                                lhsT=g2[:, c * P:(c + 1) * P],
                                rhs=S_t[:, c * P:(c + 1) * P],
                                start=(c == 0),
                                stop=(c == Cbg - 1),
                            )
                        if g == 0:
                            nc.scalar.activation(out=agg_sb[b][:], in_=agg_ps[:, :],
                                                 func=ACTF.Copy)
                            continue
                        elif g < NG - 1:
                            nc.vector.tensor_tensor(out=agg_sb[b][:], in0=agg_ps[:, :],
                                                    in1=agg_sb[b][:], op=ALU.add)
                            continue
                        agg = wpool.tile([P, P], dt.float16, tag="agg", name="agg")
                        nc.vector.tensor_tensor(out=agg[:], in0=agg_ps[:, :],
                                                in1=agg_sb[b][:], op=ALU.add)
                        finish_block(l, b, agg, Hout)

                if l < 2 and l < DBG_LAYERS - 1 and DBG_AG and not DBG_GATHERONLY:
                    for g, (r0, r1) in enumerate(row_rngs):
                        o0, o1 = int(obase[g]), int(obase[g + 1])
                        nc.gpsimd.collective_compute(
                            "AllGather",
                            mybir.AluOpType.bypass,
                            replica_groups=[list(range(NCORES))],
                            ins=[cc_in[l][r0:r1, :].opt()],
                            outs=[cc_out[l][o0:o1, :].opt()],
                        )
    nc.compile()
    return nc


def _make_in_maps(np_inputs):
    x = np.asarray(np_inputs["x"])
    edge_index = np.asarray(np_inputs["edge_index"])
    x_t, idx_arrs, S_arrs, dinv_blk, meta = _host_prep(x, edge_index)

    b1_bc = np.ascontiguousarray(np.broadcast_to(
        np.asarray(np_inputs["b1"], np.float16)[None, :], (P, H)))
    b2_bc = np.ascontiguousarray(np.broadcast_to(
        np.asarray(np_inputs["b2"], np.float16)[None, :], (P, H)))
    b3_bc = np.ascontiguousarray(np.broadcast_to(
        np.asarray(np_inputs["b3"], np.float32)[None, :], (P, O)))
    W1 = np.ascontiguousarray(np.asarray(np_inputs["W1"], np.float32).astype(np.float16))
    W2 = np.ascontiguousarray(np.asarray(np_inputs["W2"], np.float32).astype(np.float16))
    W3 = np.ascontiguousarray(np.asarray(np_inputs["W3"], np.float32).astype(np.float16))

    in_maps = []
    for c in range(NCORES):
        in_maps.append({
            "x_t": x_t,
            "idx": idx_arrs[c],
            "S": S_arrs[c],
            "dinv_blk": dinv_blk[c],
            "W1": W1, "W2": W2, "W3": W3,
            "b1_bc": b1_bc, "b2_bc": b2_bc, "b3_bc": b3_bc,
        })
    return in_maps, meta


def kernel(x, edge_index, W1, b1, W2, b2, W3, b3):
    from concourse.bass_utils import run_bass_kernel_spmd

    in_maps, meta = _make_in_maps(dict(x=x, edge_index=edge_index, W1=W1, b1=b1,
                                       W2=W2, b2=b2, W3=W3, b3=b3))
    key = ("prog", meta["CT"], tuple(meta["C"]), meta["G"], 1)
    if key not in _CACHE:
        _CACHE[key] = _build_program(meta, repeat=1)
    nc = _CACHE[key]

    res = run_bass_kernel_spmd(nc, in_maps, core_ids=list(range(NCORES)))
    out = np.concatenate([res.results[c]["out"] for c in range(NCORES)], axis=0)
    return out.astype(np.float32)


# revision 28
# speedup vs baseline: 1.5588x; 1.1755x over previous
"""3-layer GCN (message passing) on 8 Trainium2 NeuronCores.

Strategy
--------
Per GCN layer (using linearity: gcn(x) = (A_norm @ x) @ W + b):
  1. agg = A_norm @ h      -- sparse aggregate, done as per-dst-block PE matmuls
                              over dma_gather'ed source rows (fp16 table) with
                              host-built one-hot S matrices (fp8e4, segment sum).
  2. h' = ELU(agg @ W + b) -- dense GEMM per 128-node block + ELU epilogue.
Normalization dinv[src]*dinv[dst] is separable: the gather table holds
dinv-prescaled rows; dst-side dinv is applied as a per-partition ACT scale
when evicting the post-GEMM PSUM (h_ps rows are dst nodes).

Nodes are sharded contiguously across the 8 cores (6250 each); edges assigned
by destination core.  Between layers a 2-chunk AllGather republishes the fp16
node-feature table; the table is laid out CHUNK-MAJOR (chunk g = contiguous
[8 cores x local rows r0_g:r1_g]) so each AG chunk's output is contiguous and
overlaps trailing-block compute.

Gather descriptor generation is the dominant cost, so gathers are spread
round-robin over 4 SWDGE queues (num_swdge_queues=4) -- ~10x over one queue.

dma_gather indices are int16; each AG chunk's table range (<= 25600 rows) is
a gather window addressed relative to the chunk base, so per (dst block,
window) slot runs pad to 128-slot chunks uniformly across cores (single SPMD
program).
"""

import numpy as np

N = 50000
E = 800000
F = 128
H = 128
O = 64
NCORES = 8
NLOC = N // NCORES           # 6250
P = 128
NBLK = (NLOC + P - 1) // P   # 49, last block has 106 nodes
LAST_ROWS = NLOC - (NBLK - 1) * P   # 106

_CACHE = {}


def _ag_chunk_layout(ag_chunks):
    """Chunk-major table layout so each AllGather chunk's output is contiguous.

    pos(node i = c*NLOC + r) = obase[g] + c*size[g] + (r - r0[g]) where g is
    the AG chunk whose local row range [r0, r1) contains r.
    """
    bb = [round(g * NBLK / ag_chunks) for g in range(ag_chunks + 1)]
    row_rngs = [(bb[g] * P, min(bb[g + 1] * P, NLOC)) for g in range(ag_chunks)]
    sizes = [r1 - r0 for (r0, r1) in row_rngs]
    obase = np.concatenate([[0], np.cumsum(np.array(sizes) * NCORES)]).astype(np.int64)
    perm = np.empty(N, np.int64)
    r = np.arange(NLOC)
    for g, (r0, r1) in enumerate(row_rngs):
        for c in range(NCORES):
            perm[c * NLOC + r[r0:r1]] = obase[g] + c * sizes[g] + (r[r0:r1] - r0)
    return row_rngs, obase, perm


def _host_prep(x, edge_index):
    """Build per-core gather indices, S matrices, and scale vectors."""
    import os
    sfp8 = int(os.environ.get("GCN_SFP8", "1"))
    G = int(os.environ.get("GCN_AGCHUNKS", "2"))
    seqidx = int(os.environ.get("GCN_SEQIDX", "0"))
    row_rngs, obase, perm = _ag_chunk_layout(G)
    spans = [int(obase[g + 1] - obase[g]) for g in range(G)]
    assert max(spans) <= 32768

    src = np.ascontiguousarray(edge_index[0]).astype(np.int64)
    dst = np.ascontiguousarray(edge_index[1]).astype(np.int64)
    loops = np.arange(N, dtype=np.int64)
    src = np.concatenate([src, loops])
    dst = np.concatenate([dst, loops])

    deg = np.bincount(dst, minlength=N).astype(np.float64)  # includes self-loop
    dinv = (1.0 / np.sqrt(deg)).astype(np.float32)

    x_t = (dinv[:, None] * np.asarray(x, dtype=np.float32)).astype(np.float16)
    x_perm = np.empty_like(x_t)
    x_perm[perm] = x_t
    x_t = x_perm
    pos = perm[src]                     # gather addresses = table positions

    core = dst // NLOC
    ld = dst - core * NLOC
    blk = ld // P
    col = ld - blk * P

    passes = int(os.environ.get("GCN_PASSES", "1"))
    flexwin = int(os.environ.get("GCN_FLEXWIN", "0"))
    if passes == 1 and flexwin:
        # Fused blocks wait for the whole table anyway, so gather windows need
        # not align with AG chunks.  Use two overlapping windows, lo = [0,
        # 32768) and hi = [N-32768, N), with flexible assignment of the
        # overlap so per-(core, block) chunk counts stay tight and uniform.
        HI_BASE = N - 32768
        cls = np.where(pos < HI_BASE, 0,
                       np.where(pos < 32768, 1, 2)).astype(np.int64)
        key = (core * NBLK + blk) * 4 + cls
        order = np.argsort(key, kind="stable")
        pos_s = pos[order]
        key_s = key[order]
        col_s = col[order]
        cnt = np.bincount(key_s, minlength=NCORES * NBLK * 4).reshape(
            NCORES, NBLK, 4)
        n_lo, n_fx, n_hi = cnt[:, :, 0], cnt[:, :, 1], cnt[:, :, 2]
        A = np.zeros(NBLK, np.int64)
        B = np.zeros(NBLK, np.int64)
        for b in range(NBLK):
            best = None
            a_min = int(np.max(np.ceil(n_lo[:, b] / P)))
            for a in range(a_min, a_min + 3):
                spill = np.maximum(0, n_fx[:, b] - (P * a - n_lo[:, b]))
                bb2 = int(np.max(np.ceil((n_hi[:, b] + spill) / P)))
                if best is None or a + bb2 < best[0] + best[1]:
                    best = (a, bb2)
            A[b], B[b] = best
        Cg = np.stack([A, B], axis=1)                           # [NBLK, 2]
        C = Cg.sum(axis=1)
        CT = int(C.sum())
        chunk_base = np.concatenate([[0], np.cumsum(C)]).astype(np.int64)
        gbase = np.stack([chunk_base[:-1], chunk_base[:-1] + A], axis=1)
        # per-edge slot assignment with flex spill, as in the original scheme
        grp2 = key_s >> 2
        grp_cnt = np.bincount(grp2, minlength=NCORES * NBLK)
        grp_start = np.concatenate([[0], np.cumsum(grp_cnt)])
        rank = np.arange(pos_s.shape[0]) - grp_start[grp2]
        core_s = grp2 // NBLK
        blk_s = grp2 % NBLK
        k_lo = np.minimum(n_lo + n_fx, P * A[None, :])
        k_lo_e = k_lo[core_s, blk_s]
        is_lo = rank < k_lo_e
        slot_in_blk = np.where(is_lo, rank, P * A[blk_s] + (rank - k_lo_e))
        slot = chunk_base[blk_s] * P + slot_in_blk
        idx_val = np.where(is_lo, pos_s, pos_s - HI_BASE)
        win_obase = [0, HI_BASE]
        win_spans = [32768, 32768]
    else:
        grp = np.searchsorted(obase[1:-1], pos, side="right")  # source AG chunk
        key = (core * NBLK + blk) * G + grp
        order = np.argsort(key, kind="stable")
        pos_s = pos[order]
        key_s = key[order]
        col_s = col[order]
        grp_s = grp[order]
        cnt = np.bincount(key_s, minlength=NCORES * NBLK * G).reshape(
            NCORES, NBLK, G)
        Cg = np.max(np.ceil(cnt / P).astype(np.int64), axis=0)      # [NBLK, G]
        C = Cg.sum(axis=1)
        CT = int(C.sum())
        chunk_base = np.concatenate([[0], np.cumsum(C)]).astype(np.int64)
        gbase = np.zeros((NBLK, G), np.int64)
        for b in range(NBLK):
            gbase[b] = chunk_base[b] + np.concatenate([[0], np.cumsum(Cg[b])[:-1]])
        kcnt = np.bincount(key_s, minlength=NCORES * NBLK * G)
        kstart = np.concatenate([[0], np.cumsum(kcnt)])
        rank = np.arange(pos_s.shape[0]) - kstart[key_s]
        blk_s = (key_s // G) % NBLK
        core_s = key_s // (G * NBLK)
        slot = gbase[blk_s, grp_s] * P + rank
        idx_val = pos_s - obase[grp_s]
        win_obase = [int(v) for v in obase[:-1]]
        win_spans = spans
    assert idx_val.min() >= 0 and idx_val.max() < 32768

    if sfp8:
        import ml_dtypes
        s_np_dtype = ml_dtypes.float8_e4m3
    else:
        s_np_dtype = np.float16

    idx_arrs = []
    S_arrs = []
    for c in range(NCORES):
        m = core_s == c
        sl = slot[m]
        ia = np.zeros(CT * P, np.int16)
        ia[sl] = idx_val[m].astype(np.int16)
        if seqidx:  # timing control: sequential addresses (wrong results)
            ia = (np.arange(CT * P) % 16384).astype(np.int16)
        cols = CT * P // 16
        w = np.zeros((cols, 16), np.int16)
        w.reshape(-1)[:] = ia
        idx_arrs.append(np.tile(w.T.copy(), (8, 1)))
        S = np.zeros((CT, P, P), s_np_dtype)
        S[sl // P, sl % P, col_s[m]] = 1.0
        S_arrs.append(np.ascontiguousarray(S.transpose(1, 0, 2)).reshape(P, CT * P))

    dinv_blk = []
    for c in range(NCORES):
        loc = np.zeros(NBLK * P, np.float32)
        loc[:NLOC] = dinv[c * NLOC:(c + 1) * NLOC]
        dinv_blk.append(np.ascontiguousarray(loc.reshape(NBLK, P).T))

    meta = dict(Cg=Cg.tolist(), C=C.tolist(), CT=CT, gbase=gbase.tolist(),
                chunk_base=chunk_base.tolist(), sfp8=sfp8, G=G,
                row_rngs=row_rngs, obase=obase.tolist(),
                win_obase=win_obase, win_spans=win_spans, passes=passes)
    return x_t, idx_arrs, S_arrs, dinv_blk, meta


def _build_program(meta, repeat=1):
    import os
    import concourse.mybir as mybir
    import concourse.tile as tile
    from concourse import bacc

    DBG_LAYERS = int(os.environ.get("GCN_LAYERS", "3"))
    DBG_AG = int(os.environ.get("GCN_AG", "1"))
    DBG_GATHERONLY = int(os.environ.get("GCN_GATHERONLY", "0"))
    NSWQ = int(os.environ.get("GCN_NSWQ", "4"))
    PASSES = meta["passes"]

    Cg = meta["Cg"]
    C = meta["C"]
    CT = meta["CT"]
    chunk_base = meta["chunk_base"]
    gbase = meta["gbase"]
    sfp8 = meta["sfp8"]
    row_rngs = meta["row_rngs"]
    obase = meta["obase"]
    win_obase = meta["win_obase"]
    win_spans = meta["win_spans"]
    NG = len(win_spans)
    dt = mybir.dt
    s_dt = dt.float8e4 if sfp8 else dt.float16
    ALU = mybir.AluOpType
    ACTF = mybir.ActivationFunctionType

    nc = bacc.Bacc("TRN2", target_bir_lowering=False, num_devices=NCORES,
                   num_swdge_queues=NSWQ)

    t_xt = nc.dram_tensor("x_t", [N, F], dt.float16, kind="ExternalInput")
    t_idx = nc.dram_tensor("idx", [P, CT * 8], dt.int16, kind="ExternalInput")
    t_S = nc.dram_tensor("S", [P, CT * P], s_dt, kind="ExternalInput")
    t_dbk = nc.dram_tensor("dinv_blk", [P, NBLK], dt.float32, kind="ExternalInput")
    t_W = [nc.dram_tensor("W1", [F, H], dt.float16, kind="ExternalInput"),
           nc.dram_tensor("W2", [H, H], dt.float16, kind="ExternalInput"),
           nc.dram_tensor("W3", [H, O], dt.float16, kind="ExternalInput")]
    t_b = [nc.dram_tensor("b1_bc", [P, H], dt.float16, kind="ExternalInput"),
           nc.dram_tensor("b2_bc", [P, H], dt.float16, kind="ExternalInput"),
           nc.dram_tensor("b3_bc", [P, O], dt.float32, kind="ExternalInput")]
    t_out = nc.dram_tensor("out", [NLOC, O], dt.float32, kind="ExternalOutput")

    qctr = [0]

    def next_q():
        qctr[0] += 1
        return qctr[0] % NSWQ

    with tile.TileContext(nc) as tc:
        with (
            tc.tile_pool(name="const", bufs=1) as cpool,
            tc.tile_pool(name="gth", bufs=int(os.environ.get("GCN_GBUFS", "4"))) as gpool,
            tc.tile_pool(name="smat", bufs=int(os.environ.get("GCN_SBUFS", "4"))) as spool,
            tc.tile_pool(name="work", bufs=3) as wpool,
            tc.tile_pool(name="acc", bufs=1) as apool,
            tc.tile_pool(name="hout", bufs=3) as hpool,
            tc.tile_pool(name="psA", bufs=int(os.environ.get("GCN_PSABUFS", "4")), space="PSUM") as psA,
            tc.tile_pool(name="psH", bufs=2, space="PSUM") as psH,
            tc.tile_pool(name="dram", bufs=1, space="DRAM") as dpool,
        ):
            # constants
            idx_t = cpool.tile([P, CT * 8], dt.int16, tag="idx")
            nc.sync.dma_start(idx_t[:], t_idx[:, :])
            dbk_t = cpool.tile([P, NBLK], dt.float32, tag="dbk")
            nc.sync.dma_start(dbk_t[:], t_dbk[:, :])
            W_t = []
            b_t = []
            for l in range(3):
                wt = cpool.tile([128, t_W[l].shape[1]], dt.float16, tag=f"W{l}")
                nc.sync.dma_start(wt[:], t_W[l][:, :])
                W_t.append(wt)
                bt = cpool.tile([P, t_b[l].shape[1]],
                                dt.float16 if l < 2 else dt.float32, tag=f"b{l}")
                nc.sync.dma_start(bt[:], t_b[l][:, :])
                b_t.append(bt)

            # per-block partial aggregates (group partial staging)
            agg_sb = [apool.tile([P, P], dt.float32, tag=f"aggsb{b}",
                                 name=f"aggsb{b}")
                      for b in range(NBLK)]

            # inter-layer tables (internal DRAM)
            cc_in = [dpool.tile([NLOC, H], dt.float16, tag=f"ccin{l}", name=f"ccin{l}")
                     for l in range(2)]
            cc_out = [dpool.tile([N, H], dt.float16, tag=f"ccout{l}", name=f"ccout{l}")
                      for l in range(2)]

            def finish_block(l, b, agg, Hout):
                h_ps = psH.tile([P, Hout], dt.float32, tag="hps", name="h_ps")
                nc.tensor.matmul(out=h_ps[:, :], lhsT=agg[:], rhs=W_t[l][:, :],
                                 start=True, stop=True)
                rows = P if b < NBLK - 1 else LAST_ROWS
                if l < 2:
                    tmp = wpool.tile([P, Hout], dt.float16, tag="tmp", name="tmp")
                    nc.scalar.activation(out=tmp[:], in_=h_ps[:, :], func=ACTF.Copy,
                                         scale=dbk_t[:, b:b + 1])
                    t = wpool.tile([P, Hout], dt.float16, tag="t", name="t")
                    nc.vector.tensor_tensor(out=t[:], in0=tmp[:], in1=b_t[l][:, :],
                                            op=ALU.add)
                    # ELU(t) = max(exp(min(t,0)) - 1, t)
                    m = wpool.tile([P, Hout], dt.float16, tag="m", name="m")
                    nc.vector.tensor_scalar(out=m[:], in0=t[:], scalar1=0.0,
                                            scalar2=None, op0=ALU.min)
                    e = wpool.tile([P, Hout], dt.float16, tag="e", name="e")
                    nc.scalar.activation(out=e[:], in_=m[:], func=ACTF.Exp)
                    r = wpool.tile([P, Hout], dt.float16, tag="r", name="r")
                    nc.vector.tensor_scalar(out=r[:], in0=e[:], scalar1=-1.0,
                                            scalar2=None, op0=ALU.add)
                    s = wpool.tile([P, Hout], dt.float16, tag="s", name="s")
                    nc.vector.tensor_tensor(out=s[:], in0=r[:], in1=t[:], op=ALU.max)
                    # src-side dinv prescale for next layer's gather table
                    ht = hpool.tile([P, Hout], dt.float16, tag="ht", name="ht")
                    nc.scalar.activation(out=ht[:], in_=s[:], func=ACTF.Copy,
                                         scale=dbk_t[:, b:b + 1])
                    nc.sync.dma_start(cc_in[l][b * P:b * P + rows, :], ht[:rows, :])
                else:
                    tmp32 = wpool.tile([P, Hout], dt.float32, tag="tmp32", name="tmp32")
                    nc.scalar.activation(out=tmp32[:], in_=h_ps[:, :], func=ACTF.Copy,
                                         scale=dbk_t[:, b:b + 1])
                    t32 = wpool.tile([P, Hout], dt.float32, tag="t32", name="t32")
                    nc.vector.tensor_tensor(out=t32[:], in0=tmp32[:], in1=b_t[l][:, :],
                                            op=ALU.add)
                    nc.sync.dma_start(t_out[b * P:b * P + rows, :], t32[:rows, :])

            for rep in range(repeat):
              for l in range(DBG_LAYERS):
                tab = t_xt if l == 0 else cc_out[l - 1]
                Hout = H if l < 2 else O

                if PASSES == 1:
                    # fused: one pass over blocks; per block, one gather per
                    # group window, a single accumulation over all chunks.
                    for b in range(NBLK):
                        Cb = C[b]
                        cb0 = chunk_base[b]
                        g2 = gpool.tile([P, Cb * P], dt.float16, tag="g", name="g2")
                        g3 = g2[:, :].rearrange("p (c d) -> p c d", d=P)
                        for g in range(NG):
                            Cbg = Cg[b][g]
                            if Cbg == 0:
                                continue
                            c0 = gbase[b][g] - cb0
                            nc.gpsimd.dma_gather(
                                out_ap=g3[:, c0:c0 + Cbg, :],
                                in_ap=tab[win_obase[g]:win_obase[g] + win_spans[g], :],
                                idxs_ap=idx_t[:, (cb0 + c0) * 8:(cb0 + c0 + Cbg) * 8],
                                num_idxs=Cbg * P,
                                num_idxs_reg=Cbg * P,
                                elem_size=P,
                                single_packet=False,
                                queue_num=next_q(),
                            )
                        if DBG_GATHERONLY:
                            continue
                        S_t = spool.tile([P, Cb * P], s_dt, tag="S", name="S_t")
                        nc.sync.dma_start(S_t[:], t_S[:, cb0 * P:(cb0 + Cb) * P])
                        agg_ps = psA.tile([P, P], dt.float32, tag="aggps",
                                          name="agg_ps")
                        for c in range(Cb):
                            nc.tensor.matmul(
                                out=agg_ps[:, :],
                                lhsT=g2[:, c * P:(c + 1) * P],
                                rhs=S_t[:, c * P:(c + 1) * P],
                                start=(c == 0),
                                stop=(c == Cb - 1),
                            )
                        agg = wpool.tile([P, P], dt.float16, tag="agg", name="agg")
                        nc.scalar.activation(out=agg[:], in_=agg_ps[:, :],
                                             func=ACTF.Copy)
                        finish_block(l, b, agg, Hout)
                else:
                  for g in range(NG):
                    win = tab[win_obase[g]:win_obase[g] + win_spans[g], :]
                    for b in range(NBLK):
                        Cbg = Cg[b][g]
                        if Cbg == 0:
                            continue
                        cb0 = gbase[b][g]
                        g2 = gpool.tile([P, Cbg * P], dt.float16, tag=f"g{g}",
                                        name="g2")
                        g3 = g2[:, :].rearrange("p (c d) -> p c d", d=P)
                        nc.gpsimd.dma_gather(
                            out_ap=g3[:, 0:Cbg, :],
                            in_ap=win,
                            idxs_ap=idx_t[:, cb0 * 8:(cb0 + Cbg) * 8],
                            num_idxs=Cbg * P,
                            num_idxs_reg=Cbg * P,
                            elem_size=P,
                            single_packet=False,
                            queue_num=next_q(),
                        )
                        if DBG_GATHERONLY:
                            continue
                        S_t = spool.tile([P, Cbg * P], s_dt, tag=f"S{g}", name="S_t")
                        nc.sync.dma_start(S_t[:], t_S[:, cb0 * P:(cb0 + Cbg) * P])

                        agg_ps = psA.tile([P, P], dt.float32, tag="aggps",
                                          name="agg_ps")
                        for c in range(Cbg):
                            nc.tensor.matmul(
                                out=agg_ps[:, :],
                                lhsT=g2[:, c * P:(c + 1) * P],
                                rhs=S_t[:, c * P:(c + 1) * P],
                                start=(c == 0),
                                stop=(c == Cbg - 1),
                            )
                        if g == 0:
                            nc.scalar.activation(out=agg_sb[b][:], in_=agg_ps[:, :],
                                                 func=ACTF.Copy)
                            continue
                        elif g < NG - 1:
                            nc.vector.tensor_tensor(out=agg_sb[b][:], in0=agg_ps[:, :],
                                                    in1=agg_sb[b][:], op=ALU.add)
                            continue
                        agg = wpool.tile([P, P], dt.float16, tag="agg", name="agg")
                        nc.vector.tensor_tensor(out=agg[:], in0=agg_ps[:, :],
                                                in1=agg_sb[b][:], op=ALU.add)
                        finish_block(l, b, agg, Hout)

                if l < 2 and l < DBG_LAYERS - 1 and DBG_AG and not DBG_GATHERONLY:
                    for g, (r0, r1) in enumerate(row_rngs):
                        o0, o1 = int(obase[g]), int(obase[g + 1])
                        nc.gpsimd.collective_compute(
                            "AllGather",
                            mybir.AluOpType.bypass,
                            replica_groups=[list(range(NCORES))],
                            ins=[cc_in[l][r0:r1, :].opt()],
                            outs=[cc_out[l][o0:o1, :].opt()],
                        )
    nc.compile()
    return nc


def _make_in_maps(np_inputs):
    x = np.asarray(np_inputs["x"])
    edge_index = np.asarray(np_inputs["edge_index"])
    x_t, idx_arrs, S_arrs, dinv_blk, meta = _host_prep(x, edge_index)

    b1_bc = np.ascontiguousarray(np.broadcast_to(
        np.asarray(np_inputs["b1"], np.float16)[None, :], (P, H)))
    b2_bc = np.ascontiguousarray(np.broadcast_to(
        np.asarray(np_inputs["b2"], np.float16)[None, :], (P, H)))
    b3_bc = np.ascontiguousarray(np.broadcast_to(
        np.asarray(np_inputs["b3"], np.float32)[None, :], (P, O)))
    W1 = np.ascontiguousarray(np.asarray(np_inputs["W1"], np.float32).astype(np.float16))
    W2 = np.ascontiguousarray(np.asarray(np_inputs["W2"], np.float32).astype(np.float16))
    W3 = np.ascontiguousarray(np.asarray(np_inputs["W3"], np.float32).astype(np.float16))

    in_maps = []
    for c in range(NCORES):
        in_maps.append({
            "x_t": x_t,
            "idx": idx_arrs[c],
            "S": S_arrs[c],
            "dinv_blk": dinv_blk[c],
            "W1": W1, "W2": W2, "W3": W3,
            "b1_bc": b1_bc, "b2_bc": b2_bc, "b3_bc": b3_bc,
        })
    return in_maps, meta


def kernel(x, edge_index, W1, b1, W2, b2, W3, b3):
    from concourse.bass_utils import run_bass_kernel_spmd

    in_maps, meta = _make_in_maps(dict(x=x, edge_index=edge_index, W1=W1, b1=b1,
                                       W2=W2, b2=b2, W3=W3, b3=b3))
    key = ("prog", meta["CT"], tuple(meta["C"]), meta["G"], 1)
    if key not in _CACHE:
        _CACHE[key] = _build_program(meta, repeat=1)
    nc = _CACHE[key]

    res = run_bass_kernel_spmd(nc, in_maps, core_ids=list(range(NCORES)))
    out = np.concatenate([res.results[c]["out"] for c in range(NCORES)], axis=0)
    return out.astype(np.float32)
